# revision 10
# baseline (speedup 1.0000x reference)
"""Bass/Trainium2 kernel for LocalAttention (block-diagonal MHA, causal).

Model: x[B=4, SEQ=4096, D=1024] split into SPLIT=4 sequence blocks of L=1024,
each block has its own MHA weights (H=16 heads, DK=64), causal within block.

Sharding: the 16 (batch, split) blocks are embarrassingly parallel.
Core i handles split s = i//2 and batches {2*(i%2), 2*(i%2)+1}, so each core
needs exactly one split's weights. No collectives.

Per-core program (SPMD, same program for all 8 cores, different data):
  - Host pre-transposes each x-block -> Xt [d, l] so the contraction dim (d)
    lands on SBUF partitions for the projection matmuls.
  - Qt/Kt = (x W + b)^T computed with W stationary / Xt moving  -> [e, l].
    1/sqrt(DK) folded into Wq/bq on the host.
  - V computed naturally (Xt stationary / Wv moving) -> [l, e], stored bf16
    in V_aug with a ones-column per head (yields softmax denominator for
    free during the P@V matmul).
  - scores^T [lk, lq] per head (Kc=DK=64); head pairs run concurrently on
    disjoint PE row-groups (base partitions 0/64). Causal upper tiles are
    skipped; diagonal tiles use shrunk moving-N.
  - exp on ACT (no max subtraction; scores are O(1) by construction),
    writing bf16 P^T; one strided DVE multiply per head applies the
    triangular mask on the 8 diagonal 128x128 tiles (anti-diagonal AP).
  - PV: V_aug slice stationary (M=65) x P^T moving -> O^T (+ sums row 64)
    accumulated in PSUM.  sums -> DRAM -> partition-broadcast DMA ->
    reciprocal -> normalize O^T in PSUM -> DMA into Ot [e, l].
  - out-proj: Ot stationary x Wo moving -> OUT [l, e2] natural; bias
    bo' = bv @ Wo + bo (bv folded through the softmax) added via a
    broadcast-DMA'd row; result DMA'd PSUM -> DRAM.
All matmuls fp32r (full PE rate at N>=256) except P@V which is bf16.
"""

import numpy as np
import ml_dtypes
from contextlib import ExitStack

import concourse.bass as bass
import concourse.bacc as bacc
import concourse.mybir as mybir
import concourse.tile as tile
from concourse.bass_utils import run_bass_kernel_spmd

H = 16
D = 1024
SPLIT = 4
DK = 64
B = 4
SEQ = 4096
L = SEQ // SPLIT          # 1024: tokens per block
NB = 2                    # blocks per core
NCORES = 8
NT = D // 128             # 8 partition tiles of the model dim
NLT = L // 128            # 8 partition tiles of the block length
F32 = mybir.dt.float32
F32R = mybir.dt.float32r
BF16 = mybir.dt.bfloat16

# moving-N start offset for diagonal score tiles, by diagonal index d=i-4j
# (fp32r needs N>=256 to run at full rate, so d=3 backs off to N=256)
_DIAG_LO = {0: 0, 1: 128, 2: 256, 3: 256}


def _f32r(ap):
    return ap.bitcast(F32R)


def build_program():
    nc = bacc.Bacc()

    xt_d = nc.declare_dram_parameter("xt", [NB, NT, 128, L], BF16, isOutput=False)
    wq_d = nc.declare_dram_parameter("wq", [NT, NT, 128, 128], BF16, isOutput=False)
    wk_d = nc.declare_dram_parameter("wk", [NT, NT, 128, 128], BF16, isOutput=False)
    wv_d = nc.declare_dram_parameter("wv", [NT, 128, D], BF16, isOutput=False)
    wo_d = nc.declare_dram_parameter("wo", [NT, 128, D], BF16, isOutput=False)
    bq_d = nc.declare_dram_parameter("bq", [128, NT], F32, isOutput=False)
    bk_d = nc.declare_dram_parameter("bk", [128, NT], F32, isOutput=False)
    bop_d = nc.declare_dram_parameter("bop", [1, D], F32, isOutput=False)
    mask_d = nc.declare_dram_parameter("mask", [128, 128], BF16, isOutput=False)
    out_d = nc.declare_dram_parameter("out", [NB, L, D], F32, isOutput=True)

    with ExitStack() as ctx:
        tc = ctx.enter_context(tile.TileContext(nc))
        consts = ctx.enter_context(tc.tile_pool(name="consts", bufs=1))
        big = ctx.enter_context(tc.tile_pool(name="big", bufs=1))
        qk = ctx.enter_context(tc.tile_pool(name="qk", bufs=1))
        va_p = ctx.enter_context(tc.tile_pool(name="va", bufs=1))
        pt_p = ctx.enter_context(tc.tile_pool(name="pt", bufs=2))
        wstat = ctx.enter_context(tc.tile_pool(name="wstat", bufs=6))
        wmov = ctx.enter_context(tc.tile_pool(name="wmov", bufs=10))
        rec_p = ctx.enter_context(tc.tile_pool(name="rec", bufs=2))
        stg_p = ctx.enter_context(tc.tile_pool(name="stg", bufs=2))
        out_p = ctx.enter_context(tc.tile_pool(name="out", bufs=3))
        scr_p = ctx.enter_context(tc.tile_pool(name="scr", bufs=4, space="DRAM"))
        ps = ctx.enter_context(tc.tile_pool(name="ps", bufs=2, space="PSUM"))

        # constants
        bq_sb = consts.tile([128, NT], F32, tag="bq")
        bk_sb = consts.tile([128, NT], F32, tag="bk")
        mask_sb = consts.tile([128, 128], BF16, tag="mask")
        bo_bc = consts.tile([128, D], F32, tag="bobc")
        nc.sync.dma_start(out=bq_sb, in_=bq_d[:, :])
        nc.sync.dma_start(out=bk_sb, in_=bk_d[:, :])
        nc.sync.dma_start(out=mask_sb, in_=mask_d[:, :])
        bop_bcast = bass.AP(tensor=bop_d, offset=0, ap=[[0, 128], [1, D]])
        nc.gpsimd.dma_start(out=bo_bc, in_=bop_bcast)

        for blk in range(NB):
            # ---- load Xt ----------------------------------------------------
            xt = big.tile([128, NT, L], BF16, tag="xt_ot")
            nc.sync.dma_start(out=xt, in_=xt_d[blk].transpose([1, 0, 2]))

            # ---- Q/K projections (transposed out: [e, l]) -------------------
            qt = qk.tile([128, NT, L], BF16, tag="qt")
            kt = qk.tile([128, NT, L], BF16, tag="kt")
            for w_d, b_sb, o_sb, wtag in ((wq_d, bq_sb, qt, "wqt"),
                                          (wk_d, bk_sb, kt, "wkt")):
                for et in range(NT):
                    psa = ps.tile([128, 512], F32, tag="psa")
                    psb = ps.tile([128, 512], F32, tag="psb")
                    for d in range(NT):
                        wt = wstat.tile([128, 128], BF16, tag=wtag)
                        nc.sync.dma_start(out=wt, in_=w_d[d, et])
                        for j, pj in ((0, psa), (1, psb)):
                            nc.tensor.matmul(
                                pj[:, :], wt[:, :],
                                xt[:, d, j * 512:(j + 1) * 512],
                                start=(d == 0), stop=(d == NT - 1))
                    for j, pj in ((0, psa), (1, psb)):
                        nc.vector.tensor_scalar_add(
                            out=o_sb[:, et, j * 512:(j + 1) * 512],
                            in0=pj[:, :], scalar1=b_sb[:, et:et + 1])

            # ---- V projection (natural [l, e]) into V_aug bf16 --------------
            # V_aug layout: [128(lk), lk_tile 8, head 16, 65]; col 64 = ones
            va = va_p.tile([128, NLT, H, DK + 1], BF16, tag="va")
            nc.vector.memset(va[:, :, :, DK:DK + 1], 1.0)
            for g in range(2):
                chunks = []
                for d in range(NT):
                    wc = wmov.tile([128, 512], BF16, tag="wmov")
                    nc.sync.dma_start(out=wc, in_=wv_d[d, :, g * 512:(g + 1) * 512])
                    chunks.append(wc)
                for lt in range(NLT):
                    pv = ps.tile([128, 512], F32, tag="psa")
                    for d in range(NT):
                        nc.tensor.matmul(
                            pv[:, :], xt[:, d, lt * 128:(lt + 1) * 128],
                            chunks[d][:, :],
                            start=(d == 0), stop=(d == NT - 1))
                    nc.scalar.activation(
                        out=va[:, lt, g * 8:(g + 1) * 8, 0:DK],
                        in_=pv.rearrange("p (h k) -> p h k", h=8),
                        func=mybir.ActivationFunctionType.Copy)

            # ---- attention, head-pipelined ---------------------------------
            # stage A (head h): scores^T + exp + mask -> pt[h]
            # stage B (head h): PV + sums + normalize + DMA -> ot
            ot = big.tile([128, NT, L], BF16, tag="xt_ot")
            pending = None  # (pt tile, head)

            def stage_b(pt_h, h):
                et, half = h // 2, h % 2
                p0 = 64 * half
                opsums = []
                for j in range(2):
                    op = ps.tile([128, 512], F32, tag="pvs")
                    last = 4 * j + 3
                    for i in range(4 * j + 4):
                        d = i - 4 * j
                        lo = 128 * d if d > 0 else 0  # exact diagonal start
                        nc.tensor.matmul(
                            op[0:DK + 1, lo:512],
                            va[:, i, h, :],
                            pt_h[:, i, j * 512 + lo:(j + 1) * 512],
                            start=(i == 0), stop=(i == last))
                    opsums.append(op)
                # sums (psum row 64) -> SBUF -> DRAM -> broadcast -> recip.
                # rec tile: row 64 = staging for the sums row (engine ops are
                # partition-locked, so the copy out of psum must stay at
                # partition 64); rows 0:64 = broadcast reciprocal.
                rec = rec_p.tile([DK + 1, L], F32, tag="rec")
                scr = scr_p.tile([1, L], F32, tag="scr")
                for j in range(2):
                    nc.scalar.activation(
                        out=rec[DK:DK + 1, j * 512:(j + 1) * 512],
                        in_=opsums[j][DK:DK + 1, :],
                        func=mybir.ActivationFunctionType.Copy)
                nc.sync.dma_start(out=scr, in_=rec[DK:DK + 1, :])
                scr_bc = bass.AP(tensor=scr.tensor, offset=scr.offset,
                                 ap=[[0, DK], [1, L]])
                nc.gpsimd.dma_start(out=rec[0:DK, :], in_=scr_bc)
                nc.vector.reciprocal(out=rec[0:DK, :], in_=rec[0:DK, :])
                for j in range(2):
                    if half == 0:
                        # even head: rows 0:64 of the e-tile, write in place
                        nc.vector.tensor_mul(
                            out=ot[0:DK, et, j * 512:(j + 1) * 512],
                            in0=opsums[j][0:DK, :],
                            in1=rec[0:DK, j * 512:(j + 1) * 512])
                    else:
                        # odd head: normalize into staging (base 0), then DMA
                        # across partitions into rows 64:128
                        stg = stg_p.tile([DK, 512], BF16, tag="stg")
                        nc.vector.tensor_mul(
                            out=stg, in0=opsums[j][0:DK, :],
                            in1=rec[0:DK, j * 512:(j + 1) * 512])
                        nc.sync.dma_start(
                            out=ot[p0:p0 + DK, et, j * 512:(j + 1) * 512],
                            in_=stg)

            for h in range(H):
                et, half = h // 2, h % 2
                p0 = 64 * half
                pt_h = pt_p.tile([128, NLT, L], BF16, tag="pt")
                for j in range(2):
                    for i in range(4 * j + 4):
                        d = i - 4 * j
                        # score matmul start (fp32r wants moving N>=256):
                        lo_s = 128 * d if d > 0 else 0
                        # exp/PV start (exact diagonal boundary):
                        lo_v = 128 * d if d > 0 else 0
                        sps = ps.tile([128, 512], F32, tag="sps")
                        nc.tensor.matmul(
                            sps[:, lo_s:512],
                            kt[p0:p0 + DK, et, i * 128:(i + 1) * 128],
                            qt[p0:p0 + DK, et, j * 512 + lo_s:(j + 1) * 512],
                            start=True, stop=True)
                        nc.scalar.activation(
                            out=pt_h[:, i, j * 512 + lo_v:(j + 1) * 512],
                            in_=sps[:, lo_v:512],
                            func=mybir.ActivationFunctionType.Exp)
                # triangular mask on the 8 diagonal 128x128 tiles:
                # tile i sits at free offset i*1024 + (i*128 within row) = 1152*i
                diag = bass.AP(tensor=pt_h.tensor, offset=pt_h.offset,
                               ap=[pt_h.ap[0], [1152, NLT], [1, 128]])
                mask_bc = bass.AP(tensor=mask_sb.tensor, offset=mask_sb.offset,
                                  ap=[mask_sb.ap[0], [0, NLT], [1, 128]])
                nc.vector.tensor_mul(out=diag, in0=diag, in1=mask_bc)
                if pending is not None:
                    stage_b(*pending)
                pending = (pt_h, h)
            stage_b(*pending)
            pending = None

            # ---- output projection ------------------------------------------
            for g in range(2):
                chunks = []
                for et in range(NT):
                    wc = wmov.tile([128, 512], BF16, tag="wmov")
                    nc.sync.dma_start(out=wc, in_=wo_d[et, :, g * 512:(g + 1) * 512])
                    chunks.append(wc)
                for lt in range(NLT):
                    po = ps.tile([128, 512], F32, tag="psb")
                    for et in range(NT):
                        nc.tensor.matmul(
                            po[:, :], ot[:, et, lt * 128:(lt + 1) * 128],
                            chunks[et][:, :],
                            start=(et == 0), stop=(et == NT - 1))
                    osb = out_p.tile([128, 512], F32, tag="osb")
                    nc.vector.tensor_add(out=osb, in0=po,
                                         in1=bo_bc[:, g * 512:(g + 1) * 512])
                    nc.sync.dma_start(
                        out=out_d[blk, lt * 128:(lt + 1) * 128,
                                  g * 512:(g + 1) * 512],
                        in_=osb)
    nc.compile()
    return nc


def _prep_core_inputs(core, x, Wq, Wk, Wv, Wo, bq, bk, bv, bo, mask_bf16):
    s = core // 2
    bs = (2 * (core % 2), 2 * (core % 2) + 1)
    sc = np.float32(1.0 / np.sqrt(DK))
    xt = np.empty((NB, NT, 128, L), ml_dtypes.bfloat16)
    for n, b in enumerate(bs):
        xt[n] = np.ascontiguousarray(x[b, s * L:(s + 1) * L, :].T).reshape(NT, 128, L).astype(ml_dtypes.bfloat16)
    wq = np.ascontiguousarray(
        (Wq[s] * sc).reshape(NT, 128, NT, 128).transpose(0, 2, 1, 3)).astype(ml_dtypes.bfloat16)
    wk = np.ascontiguousarray(
        Wk[s].reshape(NT, 128, NT, 128).transpose(0, 2, 1, 3)).astype(ml_dtypes.bfloat16)
    wv = np.ascontiguousarray(Wv[s].reshape(NT, 128, D)).astype(ml_dtypes.bfloat16)
    wo = np.ascontiguousarray(Wo[s].reshape(NT, 128, D)).astype(ml_dtypes.bfloat16)
    bqt = np.ascontiguousarray((bq[s] * sc).reshape(NT, 128).T)
    bkt = np.ascontiguousarray(bk[s].reshape(NT, 128).T)
    bop = (bv[s] @ Wo[s] + bo[s]).reshape(1, D).astype(np.float32)
    return {"xt": xt, "wq": wq, "wk": wk, "wv": wv, "wo": wo,
            "bq": bqt, "bk": bkt, "bop": bop, "mask": mask_bf16}


_PROGRAM_CACHE = {}


def run(x, Wq, Wk, Wv, Wo, bq, bk, bv, bo, trace=False, **run_kwargs):
    x = np.asarray(x, np.float32)
    Wq, Wk, Wv, Wo = (np.asarray(a, np.float32) for a in (Wq, Wk, Wv, Wo))
    bq, bk, bv, bo = (np.asarray(a, np.float32) for a in (bq, bk, bv, bo))
    mask_bf16 = np.triu(np.ones((128, 128))).astype(ml_dtypes.bfloat16)

    if "nc" not in _PROGRAM_CACHE:
        _PROGRAM_CACHE["nc"] = build_program()
    nc = _PROGRAM_CACHE["nc"]

    in_maps = [_prep_core_inputs(c, x, Wq, Wk, Wv, Wo, bq, bk, bv, bo, mask_bf16)
               for c in range(NCORES)]
    res = run_bass_kernel_spmd(nc, in_maps, core_ids=list(range(NCORES)),
                               trace=trace, **run_kwargs)
    out = np.empty((B, SEQ, D), np.float32)
    for c in range(NCORES):
        s = c // 2
        for n, b in enumerate((2 * (c % 2), 2 * (c % 2) + 1)):
            out[b, s * L:(s + 1) * L, :] = res.results[c]["out"][n]
    return out, res


def kernel(x, Wq, Wk, Wv, Wo, bq, bk, bv, bo):
    out, _ = run(x, Wq, Wk, Wv, Wo, bq, bk, bv, bo, trace=False)
    return out


# revision 24
# speedup vs baseline: 113.9964x; 113.9964x over previous
"""Bass/Trainium2 kernel for LocalAttention (block-diagonal MHA, causal).

Model: x[B=4, SEQ=4096, D=1024] split into SPLIT=4 sequence blocks of L=1024,
each block has its own MHA weights (H=16 heads, DK=64), causal within block.

Sharding: the 16 (batch, split) blocks are embarrassingly parallel.
Core i handles split s = i//2 and batches {2*(i%2), 2*(i%2)+1}, so each core
needs exactly one split's weights. No collectives.

Per-core program (SPMD, same program for all 8 cores, different data):
  - Host pre-transposes each x-block -> Xt [d, l] so the contraction dim (d)
    lands on SBUF partitions for the projection matmuls.
  - Qt/Kt = (x W + b)^T computed with W stationary / Xt moving  -> [e, l].
    1/sqrt(DK) folded into Wq/bq on the host.
  - V computed naturally (Xt stationary / Wv moving) -> [l, e], stored bf16
    in V_aug with a ones-column per head (yields softmax denominator for
    free during the P@V matmul).
  - scores^T [lk, lq] per head (Kc=DK=64); head pairs run concurrently on
    disjoint PE row-groups (base partitions 0/64). Causal upper tiles are
    skipped; diagonal tiles use shrunk moving-N.
  - exp on ACT (no max subtraction; scores are O(1) by construction),
    writing bf16 P^T; one strided DVE multiply per head applies the
    triangular mask on the 8 diagonal 128x128 tiles (anti-diagonal AP).
  - PV: V_aug slice stationary (M=65) x P^T moving -> O^T (+ sums row 64)
    accumulated in PSUM.  sums -> DRAM -> partition-broadcast DMA ->
    reciprocal -> normalize O^T in PSUM -> DMA into Ot [e, l].
  - out-proj: Ot stationary x Wo moving -> OUT [l, e2] natural; bias
    bo' = bv @ Wo + bo (bv folded through the softmax) added via a
    broadcast-DMA'd row; result DMA'd PSUM -> DRAM.
All matmuls fp32r (full PE rate at N>=256) except P@V which is bf16.
"""

import numpy as np
import ml_dtypes
from contextlib import ExitStack

import concourse.bass as bass
import concourse.bacc as bacc
import concourse.mybir as mybir
import concourse.tile as tile
from concourse.bass_utils import run_bass_kernel_spmd

H = 16
D = 1024
SPLIT = 4
DK = 64
B = 4
SEQ = 4096
L = SEQ // SPLIT          # 1024: tokens per block
NB = 2                    # blocks per core
NCORES = 8
NT = D // 128             # 8 partition tiles of the model dim
NLT = L // 128            # 8 partition tiles of the block length
F32 = mybir.dt.float32
F32R = mybir.dt.float32r
BF16 = mybir.dt.bfloat16

# moving-N start offset for diagonal score tiles, by diagonal index d=i-4j
# (fp32r needs N>=256 to run at full rate, so d=3 backs off to N=256)
_DIAG_LO = {0: 0, 1: 128, 2: 256, 3: 256}


def _f32r(ap):
    return ap.bitcast(F32R)


def build_program():
    nc = bacc.Bacc()

    xt_d = nc.declare_dram_parameter("xt", [NB, NT, 128, L], BF16, isOutput=False)
    wq_d = nc.declare_dram_parameter("wq", [NT, NT, 128, 128], BF16, isOutput=False)
    wk_d = nc.declare_dram_parameter("wk", [NT, NT, 128, 128], BF16, isOutput=False)
    wv_d = nc.declare_dram_parameter("wv", [NT, 128, D], BF16, isOutput=False)
    wo_d = nc.declare_dram_parameter("wo", [NT, 128, D], BF16, isOutput=False)
    bq_d = nc.declare_dram_parameter("bq", [128, NT], F32, isOutput=False)
    bk_d = nc.declare_dram_parameter("bk", [128, NT], F32, isOutput=False)
    bop_d = nc.declare_dram_parameter("bop", [1, D], F32, isOutput=False)
    mask_d = nc.declare_dram_parameter("mask", [128, 128], BF16, isOutput=False)
    out_d = nc.declare_dram_parameter("out", [NB, L, D], F32, isOutput=True)

    with ExitStack() as ctx:
        tc = ctx.enter_context(tile.TileContext(nc))
        consts = ctx.enter_context(tc.tile_pool(name="consts", bufs=1))
        big = ctx.enter_context(tc.tile_pool(name="big", bufs=2))
        qk = ctx.enter_context(tc.tile_pool(name="qk", bufs=2))
        va_p = ctx.enter_context(tc.tile_pool(name="va", bufs=1))
        pt_p = ctx.enter_context(tc.tile_pool(name="pt", bufs=2))
        wstat = ctx.enter_context(tc.tile_pool(name="wstat", bufs=6))
        wmov = ctx.enter_context(tc.tile_pool(name="wmov", bufs=10))
        rec_p = ctx.enter_context(tc.tile_pool(name="rec", bufs=2))
        stg_p = ctx.enter_context(tc.tile_pool(name="stg", bufs=2))
        out_p = ctx.enter_context(tc.tile_pool(name="out", bufs=3))
        scr_p = ctx.enter_context(tc.tile_pool(name="scr", bufs=4, space="DRAM"))
        ps = ctx.enter_context(tc.tile_pool(name="ps", bufs=3, space="PSUM"))

        # constants
        bq_sb = consts.tile([128, NT], F32, tag="bq")
        bk_sb = consts.tile([128, NT], F32, tag="bk")
        mask_sb = consts.tile([128, 128], BF16, tag="mask")
        bo_bc = consts.tile([128, D], F32, tag="bobc")
        nc.sync.dma_start(out=bq_sb, in_=bq_d[:, :])
        nc.sync.dma_start(out=bk_sb, in_=bk_d[:, :])
        nc.sync.dma_start(out=mask_sb, in_=mask_d[:, :])
        bop_bcast = bass.AP(tensor=bop_d, offset=0, ap=[[0, 128], [1, D]])
        nc.gpsimd.dma_start(out=bo_bc, in_=bop_bcast)

        for blk in range(NB):
            # ---- load Xt ----------------------------------------------------
            xt = big.tile([128, NT, L], BF16, tag="xt_ot")
            nc.sync.dma_start(out=xt, in_=xt_d[blk].transpose([1, 0, 2]))

            # ---- Q/K projections (transposed out: [e, l]) -------------------
            qt = qk.tile([128, NT, L], BF16, tag="qt")
            kt = qk.tile([128, NT, L], BF16, tag="kt")
            for w_d, b_sb, o_sb, wtag in ((wq_d, bq_sb, qt, "wqt"),
                                          (wk_d, bk_sb, kt, "wkt")):
                for et in range(NT):
                    psa = ps.tile([128, 512], F32, tag="psa", bufs=3)
                    psb = ps.tile([128, 512], F32, tag="psa", bufs=3)
                    for d in range(NT):
                        wt = wstat.tile([128, 128], BF16, tag=wtag)
                        nc.sync.dma_start(out=wt, in_=w_d[d, et])
                        for j, pj in ((0, psa), (1, psb)):
                            nc.tensor.matmul(
                                pj[:, :], wt[:, :],
                                xt[:, d, j * 512:(j + 1) * 512],
                                start=(d == 0), stop=(d == NT - 1))
                    for j, pj in ((0, psa), (1, psb)):
                        nc.vector.tensor_scalar_add(
                            out=o_sb[:, et, j * 512:(j + 1) * 512],
                            in0=pj[:, :], scalar1=b_sb[:, et:et + 1])

            # ---- V projection (natural [l, e]) into V_aug bf16 --------------
            # V_aug layout: [128(lk), lk_tile 8, head 16, 65]; col 64 = ones
            va = va_p.tile([128, NLT, H, DK + 1], BF16, tag="va")
            nc.vector.memset(va[:, :, :, DK:DK + 1], 1.0)
            for g in range(2):
                chunks = []
                for d in range(NT):
                    wc = wmov.tile([128, 512], BF16, tag="wmov")
                    nc.sync.dma_start(out=wc, in_=wv_d[d, :, g * 512:(g + 1) * 512])
                    chunks.append(wc)
                for lt in range(NLT):
                    pv = ps.tile([128, 512], F32, tag="psa", bufs=3)
                    for d in range(NT):
                        nc.tensor.matmul(
                            pv[:, :], xt[:, d, lt * 128:(lt + 1) * 128],
                            chunks[d][:, :],
                            start=(d == 0), stop=(d == NT - 1))
                    nc.scalar.activation(
                        out=va[:, lt, g * 8:(g + 1) * 8, 0:DK],
                        in_=pv.rearrange("p (h k) -> p h k", h=8),
                        func=mybir.ActivationFunctionType.Copy)

            # ---- attention, head-pipelined ---------------------------------
            # stage A (head h): scores^T + exp + mask -> pt[h]
            # stage B (head h): PV + sums + normalize + DMA -> ot
            ot = big.tile([128, NT, L], BF16, tag="xt_ot")
            pending = None  # (pt tile, head)

            def stage_b(pt_h, h):
                et, half = h // 2, h % 2
                p0 = 64 * half
                opsums = []
                for j in range(2):
                    op = ps.tile([128, 512], F32, tag="pvs")
                    last = 4 * j + 3
                    for i in range(4 * j + 4):
                        d = i - 4 * j
                        lo = 128 * d if d > 0 else 0  # exact diagonal start
                        nc.tensor.matmul(
                            op[0:DK + 1, lo:512],
                            va[:, i, h, :],
                            pt_h[j][:, i, lo:512],
                            start=(i == 0), stop=(i == last))
                    opsums.append(op)
                # sums (psum row 64) -> SBUF -> DRAM -> broadcast -> recip.
                # rec tile: row 64 = staging for the sums row (engine ops are
                # partition-locked, so the copy out of psum must stay at
                # partition 64); rows 0:64 = broadcast reciprocal.
                rec = rec_p.tile([DK + 1, L], F32, tag="rec")
                scr = scr_p.tile([1, L], F32, tag="scr")
                for j in range(2):
                    nc.vector.tensor_copy(
                        out=rec[DK:DK + 1, j * 512:(j + 1) * 512],
                        in_=opsums[j][DK:DK + 1, :])
                nc.sync.dma_start(out=scr, in_=rec[DK:DK + 1, :])
                scr_bc = bass.AP(tensor=scr.tensor, offset=scr.offset,
                                 ap=[[0, DK], [1, L]])
                nc.gpsimd.dma_start(out=rec[0:DK, :], in_=scr_bc)
                nc.vector.reciprocal(out=rec[0:DK, :], in_=rec[0:DK, :])
                for j in range(2):
                    if half == 0:
                        # even head: rows 0:64 of the e-tile, write in place
                        nc.vector.tensor_mul(
                            out=ot[0:DK, et, j * 512:(j + 1) * 512],
                            in0=opsums[j][0:DK, :],
                            in1=rec[0:DK, j * 512:(j + 1) * 512])
                    else:
                        # odd head: normalize into staging (base 0), then DMA
                        # across partitions into rows 64:128
                        stg = stg_p.tile([DK, 512], BF16, tag="stg")
                        nc.vector.tensor_mul(
                            out=stg, in0=opsums[j][0:DK, :],
                            in1=rec[0:DK, j * 512:(j + 1) * 512])
                        nc.sync.dma_start(
                            out=ot[p0:p0 + DK, et, j * 512:(j + 1) * 512],
                            in_=stg)

            for h in range(H):
                et, half = h // 2, h % 2
                p0 = 64 * half
                # per-j P^T tiles: PV(j) only depends on its own exps
                pt_h = (pt_p.tile([128, 4, 512], BF16, name="pt0", tag="pt0", bufs=3),
                        pt_p.tile([128, NLT, 512], BF16, name="pt1", tag="pt1", bufs=3))
                for j in range(2):
                    for i in range(4 * j + 4):
                        d = i - 4 * j
                        lo_v = 128 * d if d > 0 else 0
                        sps = ps.tile([128, 512], F32, tag="sps", bufs=2)
                        nc.tensor.matmul(
                            sps[:, lo_v:512],
                            kt[p0:p0 + DK, et, i * 128:(i + 1) * 128],
                            qt[p0:p0 + DK, et, j * 512 + lo_v:(j + 1) * 512],
                            start=True, stop=True)
                        nc.scalar.activation(
                            out=pt_h[j][:, i, lo_v:512],
                            in_=sps[:, lo_v:512],
                            func=mybir.ActivationFunctionType.Exp)
                    # triangular mask: within pt_h[j], diagonal tile i sits at
                    # free offset i*512 + 128*(i-4j) = 640*i - 2048*j
                    pt = pt_h[j]
                    diag = bass.AP(tensor=pt.tensor,
                                   offset=pt.offset + 2048 * j,
                                   ap=[pt.ap[0], [640, 4], [1, 128]])
                    mask_bc = bass.AP(tensor=mask_sb.tensor,
                                      offset=mask_sb.offset,
                                      ap=[mask_sb.ap[0], [0, 4], [1, 128]])
                    nc.vector.tensor_mul(out=diag, in0=diag, in1=mask_bc)
                if pending is not None:
                    stage_b(*pending)
                pending = (pt_h, h)
            stage_b(*pending)
            pending = None

            # ---- output projection ------------------------------------------
            for g in range(2):
                chunks = []
                for et in range(NT):
                    wc = wmov.tile([128, 512], BF16, tag="wmov")
                    nc.sync.dma_start(out=wc, in_=wo_d[et, :, g * 512:(g + 1) * 512])
                    chunks.append(wc)
                for lt in range(NLT):
                    po = ps.tile([128, 512], F32, tag="psa", bufs=3)
                    for et in range(NT):
                        nc.tensor.matmul(
                            po[:, :], ot[:, et, lt * 128:(lt + 1) * 128],
                            chunks[et][:, :],
                            start=(et == 0), stop=(et == NT - 1))
                    osb = out_p.tile([128, 512], F32, tag="osb")
                    nc.vector.tensor_add(out=osb, in0=po,
                                         in1=bo_bc[:, g * 512:(g + 1) * 512])
                    nc.sync.dma_start(
                        out=out_d[blk, lt * 128:(lt + 1) * 128,
                                  g * 512:(g + 1) * 512],
                        in_=osb)
    nc.compile()
    return nc


def _prep_core_inputs(core, x, Wq, Wk, Wv, Wo, bq, bk, bv, bo, mask_bf16):
    s = core // 2
    bs = (2 * (core % 2), 2 * (core % 2) + 1)
    sc = np.float32(1.0 / np.sqrt(DK))
    xt = np.empty((NB, NT, 128, L), ml_dtypes.bfloat16)
    for n, b in enumerate(bs):
        xt[n] = np.ascontiguousarray(x[b, s * L:(s + 1) * L, :].T).reshape(NT, 128, L).astype(ml_dtypes.bfloat16)
    wq = np.ascontiguousarray(
        (Wq[s] * sc).reshape(NT, 128, NT, 128).transpose(0, 2, 1, 3)).astype(ml_dtypes.bfloat16)
    wk = np.ascontiguousarray(
        Wk[s].reshape(NT, 128, NT, 128).transpose(0, 2, 1, 3)).astype(ml_dtypes.bfloat16)
    wv = np.ascontiguousarray(Wv[s].reshape(NT, 128, D)).astype(ml_dtypes.bfloat16)
    wo = np.ascontiguousarray(Wo[s].reshape(NT, 128, D)).astype(ml_dtypes.bfloat16)
    bqt = np.ascontiguousarray((bq[s] * sc).reshape(NT, 128).T)
    bkt = np.ascontiguousarray(bk[s].reshape(NT, 128).T)
    bop = (bv[s] @ Wo[s] + bo[s]).reshape(1, D).astype(np.float32)
    return {"xt": xt, "wq": wq, "wk": wk, "wv": wv, "wo": wo,
            "bq": bqt, "bk": bkt, "bop": bop, "mask": mask_bf16}


_PROGRAM_CACHE = {}


def run(x, Wq, Wk, Wv, Wo, bq, bk, bv, bo, trace=False, **run_kwargs):
    x = np.asarray(x, np.float32)
    Wq, Wk, Wv, Wo = (np.asarray(a, np.float32) for a in (Wq, Wk, Wv, Wo))
    bq, bk, bv, bo = (np.asarray(a, np.float32) for a in (bq, bk, bv, bo))
    mask_bf16 = np.triu(np.ones((128, 128))).astype(ml_dtypes.bfloat16)

    if "nc" not in _PROGRAM_CACHE:
        _PROGRAM_CACHE["nc"] = build_program()
    nc = _PROGRAM_CACHE["nc"]

    in_maps = [_prep_core_inputs(c, x, Wq, Wk, Wv, Wo, bq, bk, bv, bo, mask_bf16)
               for c in range(NCORES)]
    res = run_bass_kernel_spmd(nc, in_maps, core_ids=list(range(NCORES)),
                               trace=trace, **run_kwargs)
    out = np.empty((B, SEQ, D), np.float32)
    for c in range(NCORES):
        s = c // 2
        for n, b in enumerate((2 * (c % 2), 2 * (c % 2) + 1)):
            out[b, s * L:(s + 1) * L, :] = res.results[c]["out"][n]
    return out, res


def kernel(x, Wq, Wk, Wv, Wo, bq, bk, bv, bo):
    out, _ = run(x, Wq, Wk, Wv, Wo, bq, bk, bv, bo, trace=False)
    return out
